# revision 1
# baseline (speedup 1.0000x reference)
"""Trainium2 Bass kernel: batched Ising energies E_b = s_b^T J s_b.

state: [1024, 2048] float32 in {0,1};  J: [2048, 2048] float32.
Returns energies [1024] float32.

Strategy (8 NeuronCores): sharding is 2D, 4 column-blocks of J x 2
batch-halves.  Core (r, c) computes, for its batch half and J block,
partial_rc[b] = sum_{j in cols_r} (spins[b,:] @ J[:, j]) * spins[b, j]
via PE matmuls (contraction over all 2048 rows of J) plus a
multiply+reduce on the vector engine.  The host sums the 4 column-block
partials per batch half - no on-device collectives.

J precision modes:
  "f32r": J streamed as FP32R (fp32 truncated to ~FP22 inside the PE,
          full matmul rate for moving dim >= 256).  state ships as
          uint8 and is expanded to +-1 fp32 spins on the otherwise-idle
          vector engine (PE matmul requires both operands 32-bit).
  "hilo": J = bf16 hi + bf16 lo, two accumulated matmul passes
          (fp32-level accuracy, 2x matmul work)
  "hi":   J as bf16 only (fastest, bf16-level accuracy)

All device inputs are pre-arranged on the host into [128, X] layouts that
are contiguous per SBUF partition, so DMA descriptors are 4-16KB and the
loads run near HBM rate on a single HWDGE ring in exact consumption
order.  A burst of small dummy matmuls on a zeroed tile warms the PE
clock gate (HAM) while the loads are in flight.
"""

import sys

if "/opt/trn_rl_repo" not in sys.path:
    sys.path.insert(0, "/opt/trn_rl_repo")

import numpy as np
import ml_dtypes

B, N = 1024, 2048
R, C = 4, 2          # J column-block split x batch split (R*C = 8 cores)
CB = N // R          # 512 J-columns per core
BH = B // C          # 512 samples per core
P = 128
KT = N // P          # 16 contraction tiles
BT = BH // P         # 4 output-partition tiles
CHUNK = 4            # k-tiles per input DMA
N_WARM = 150         # small dummy matmuls to warm the PE clock gate
WARM_N = 64          # free dim of each warmup matmul
K_TAIL = 4           # k-tiles computed per-b at the end (epilogue stagger)
MODE = "f32r"        # "f32r" | "hilo" | "hi"

_cache = {}


def _build_program():
    import concourse.bacc as bacc
    import concourse.mybir as mybir
    import concourse.tile as tile

    bf16 = mybir.dt.bfloat16
    f32 = mybir.dt.float32
    f32r = mybir.dt.float32r
    u8 = mybir.dt.uint8
    jdt = f32r if MODE == "f32r" else bf16

    nc = bacc.Bacc("TRN2", target_bir_lowering=False, debug=False, num_devices=R * C)

    if MODE == "f32r":
        su_ext = nc.dram_tensor("su", [P, KT * BH], u8, kind="ExternalInput").ap()
    else:
        st_ext = nc.dram_tensor("st", [P, KT * BH], bf16, kind="ExternalInput").ap()
    jhi_ext = nc.dram_tensor("jhi", [P, KT * CB], jdt, kind="ExternalInput").ap()
    jlo_ext = (
        nc.dram_tensor("jlo", [P, KT * CB], bf16, kind="ExternalInput").ap()
        if MODE == "hilo"
        else None
    )
    sb_ext = nc.dram_tensor("sb", [P, BT * CB], bf16, kind="ExternalInput").ap()
    out_ext = nc.dram_tensor("part", [BH], f32, kind="ExternalOutput").ap()

    with tile.TileContext(nc) as tc:
        with (
            tc.tile_pool(name="persist", bufs=1) as persist,
            tc.tile_pool(name="work", bufs=3) as work,
            tc.tile_pool(name="psum", bufs=1, space="PSUM") as psum_pool,
            tc.tile_pool(name="warmps", bufs=1, space="PSUM") as warm_pool,
        ):
            sdt = f32r if MODE == "f32r" else bf16
            st_t = persist.tile([P, KT, BH], sdt)
            su_t = (
                persist.tile([P, KT, BH], u8, name="su_t")
                if MODE == "f32r"
                else None
            )
            jhi_t = persist.tile([P, KT, CB], jdt)
            jlo_t = (
                persist.tile([P, KT, CB], bf16, name="jlo_t")
                if MODE == "hilo"
                else None
            )
            sb_t = persist.tile([P, BT, CB], bf16)
            red_all = persist.tile([P, BT], f32)
            warm_src = persist.tile([P, CB], bf16)

            # PE warmup: small dummy matmuls on a zeroed tile keep the HAM
            # activity window busy while the real loads stream in.  Small
            # free dim => the last one never delays the first real matmul.
            nc.vector.memset(warm_src[:], 0.0)
            warm_ps = warm_pool.tile([P, WARM_N], f32)
            for _ in range(N_WARM):
                nc.tensor.matmul(
                    warm_ps, lhsT=warm_src[:, :P], rhs=warm_src[:, :WARM_N],
                    start=True, stop=True,
                )

            # Input loads: chunks with 4-16KB per-partition contiguous
            # runs.
            n_chunks = KT // CHUNK
            if MODE == "f32r":

                def jchunk(k0, k1, eng):
                    eng.dma_start(
                        out=jhi_t[:, k0:k1],
                        in_=jhi_ext[:, k0 * CB : k1 * CB],
                    )

                # All loads ride ONE ring (sync: earlier first byte) in
                # exact consumption order, so arrival order is
                # deterministic - splitting across the two HWDGE rings
                # makes the per-ring share flap with queued bytes and
                # starves whichever ring holds the next-needed chunk.
                # Geometric head chunks: each DMA pays a ~3.4us
                # completion-to-consumer latency, so a tiny first chunk
                # lets matmuls start early while later, larger chunks'
                # latencies pipeline behind compute.
                su3 = su_ext.rearrange("p (k b) -> p k b", b=BH)
                nc.sync.dma_start(out=su_t[:, :CHUNK], in_=su3[:, :CHUNK])
                jchunk(0, 1, nc.sync)
                jchunk(1, 2, nc.sync)
                jchunk(2, 3, nc.sync)
                nc.sync.dma_start(out=su_t[:, CHUNK:], in_=su3[:, CHUNK:])
                bounds = [3, 4, 6, 8, 10, 12, 16]
                for ci in range(len(bounds) - 1):
                    jchunk(bounds[ci], bounds[ci + 1], nc.sync)
                # expand uint8 {0,1} -> +-1.0 spins on the vector engine
                for ci in range(n_chunks):
                    kt = slice(ci * CHUNK, (ci + 1) * CHUNK)
                    nc.vector.tensor_scalar(
                        st_t[:, kt],
                        su_t[:, kt],
                        2.0,
                        -1.0,
                        mybir.AluOpType.mult,
                        mybir.AluOpType.add,
                    )
            else:
                for ci in range(n_chunks):
                    kt = slice(ci * CHUNK, (ci + 1) * CHUNK)
                    ks = slice(ci * CHUNK * BH, (ci + 1) * CHUNK * BH)
                    kc = slice(ci * CHUNK * CB, (ci + 1) * CHUNK * CB)
                    nc.sync.dma_start(out=st_t[:, kt], in_=st_ext[:, ks])
                    nc.scalar.dma_start(out=jhi_t[:, kt], in_=jhi_ext[:, kc])
            sb_eng = nc.sync
            sb_eng.dma_start(out=sb_t[:], in_=sb_ext.rearrange(
                "p (t c) -> p t c", c=CB))
            if MODE == "hilo":
                for ci in range(n_chunks):
                    kt = slice(ci * CHUNK, (ci + 1) * CHUNK)
                    kc = slice(ci * CHUNK * CB, (ci + 1) * CHUNK * CB)
                    eng = nc.sync if ci % 2 == 0 else nc.scalar
                    eng.dma_start(out=jlo_t[:, kt], in_=jlo_ext[:, kc])

            ps_tiles = [
                psum_pool.tile([P, CB], f32, name=f"ps_{b}") for b in range(BT)
            ]

            def mm(b, k, jt, start, stop):
                nc.tensor.matmul(
                    ps_tiles[b],
                    lhsT=st_t[:, k, b * P : (b + 1) * P],
                    rhs=jt[:, k],
                    start=start,
                    stop=stop,
                )

            out3 = out_ext.rearrange("(t p) -> p t", p=P)

            def epilogue(b):
                m = work.tile([P, CB], f32, name="m_epi")
                nc.vector.scalar_tensor_tensor(
                    m[:],
                    ps_tiles[b][:],
                    1.0,
                    sb_t[:, b],
                    mybir.AluOpType.mult,
                    mybir.AluOpType.mult,
                    accum_out=red_all[:, b : b + 1],
                )
                # per-b output DMA: the first three hide under the
                # remaining matmuls, only the last is exposed
                nc.sync.dma_start(out=out3[:, b : b + 1], in_=red_all[:, b : b + 1])

            if MODE == "hilo":
                # hi pass k-outer (4 matmuls runnable per arriving chunk),
                # then lo pass b-outer so epilogues overlap remaining MMs
                for k in range(KT):
                    for b in range(BT):
                        mm(b, k, jhi_t, start=(k == 0), stop=False)
                for b in range(BT):
                    for k in range(KT):
                        mm(b, k, jlo_t, start=False, stop=(k == KT - 1))
                    epilogue(b)
            else:
                # single pass: k-outer for the bulk, the last K_TAIL
                # k-tiles go b-by-b so epilogues overlap the tail matmuls
                for k in range(KT - K_TAIL):
                    for b in range(BT):
                        mm(b, k, jhi_t, start=(k == 0), stop=False)
                for b in range(BT):
                    for k in range(KT - K_TAIL, KT):
                        mm(b, k, jhi_t, start=False, stop=(k == KT - 1))
                    epilogue(b)


    nc.compile()
    return nc


def _part_layout(a, inner):
    """[KT*P, inner] row-major -> [P, KT*inner] contiguous per partition."""
    k = a.shape[0] // P
    return np.ascontiguousarray(
        a.reshape(k, P, inner).transpose(1, 0, 2).reshape(P, k * inner)
    )


def _make_in_maps(state, J):
    bf16 = ml_dtypes.bfloat16
    state = np.asarray(state, dtype=np.float32)
    J = np.asarray(J, dtype=np.float32)

    spins = state * 2.0 - 1.0                       # exact in fp32
    sp_bf = spins.astype(bf16)                      # [B, N], exact (+-1)
    if MODE == "f32r":
        su_all = state.astype(np.uint8).T           # [N, B] {0,1}
        Jhi = J
    else:
        st_all = sp_bf.T                            # [N, B] view
        Jhi = J.astype(bf16)
        if MODE == "hilo":
            Jlo = (J - Jhi.astype(np.float32)).astype(bf16)

    in_maps = []
    placement = []
    for core in range(R * C):
        r, c = divmod(core, C)
        m = {
            "jhi": _part_layout(Jhi[:, r * CB : (r + 1) * CB], CB),
            "sb": _part_layout(
                sp_bf[c * BH : (c + 1) * BH, r * CB : (r + 1) * CB], CB
            ),
        }
        if MODE == "f32r":
            m["su"] = _part_layout(su_all[:, c * BH : (c + 1) * BH], BH)
        else:
            m["st"] = _part_layout(st_all[:, c * BH : (c + 1) * BH], BH)
        if MODE == "hilo":
            m["jlo"] = _part_layout(Jlo[:, r * CB : (r + 1) * CB], CB)
        in_maps.append(m)
        placement.append((r, c))
    return in_maps, placement


def kernel(state, J):
    from concourse.bass_utils import run_bass_kernel_spmd

    if "nc" not in _cache:
        _cache["nc"] = _build_program()
    nc = _cache["nc"]

    in_maps, placement = _make_in_maps(state, J)
    res = run_bass_kernel_spmd(nc, in_maps, list(range(R * C)))

    out = np.zeros(B, dtype=np.float32)
    for core, (r, c) in enumerate(placement):
        out[c * BH : (c + 1) * BH] += res.results[core]["part"]
    return out



# revision 3
# speedup vs baseline: 1.3110x; 1.3110x over previous
"""Trainium2 Bass kernel: batched Ising energies E_b = s_b^T J s_b.

state: [1024, 2048] float32 in {0,1};  J: [2048, 2048] float32.
Returns energies [1024] float32.

Strategy (8 NeuronCores): symmetric-half circulant decomposition.
With A = J + J^T, E_b = 1/2 s_b^T A s_b needs only the 136 distinct
128x128 block-pairs {(p,q): p<=q} of the 16x16 block grid instead of
all 256.  A rotational starter splits those pairs into 4 isomorphic
34-block templates: core c (c = 0..3) owns block-columns
{c, 4+c, 8+c, 12+c}; column 4k+c accumulates contributions from
p = (4k+c+d) mod 16 for d = 0..7 (+ d = 8 for k = 0,1), i.e. 9/9/8/8
blocks.  Every core therefore runs the IDENTICAL instruction stream;
only the data (J blocks, spin-block permutation) differs.  The batch
is halved across the remaining factor of 2 (8 = 4 templates x 2).

Per-core compute: for column q, ps[qcol, b] = sum_p W_pq^T spins_p
via 8-9 accumulating PE matmuls (lhsT = 128x128 J block, rhs =
spins_p^T [128, 512 samples], full 512 moving dim).  Epilogue:
m = ps * spins_q^T elementwise on the vector engine (spins^T is the
same layout as the streamed state, so no second spin tensor is
shipped), then a ones-vector matmul folds the 128 partition rows into
a [1, 512] per-sample partial.  One 2KB output DMA per core; the host
sums the 4 template partials per batch half.

Inputs ship as: state as uint8 {0,1} in [128, 16 slots, 512] spin-major
layout (expanded to +-1 bf16 spins on the vector engine), J blocks as
bf16 [128, 34, 128] in exact consumption order.  All loads ride one
HWDGE ring interleaved su-chunk / J-chunk so the first matmul can
start as soon as slot 0 and block 0 have landed.  A short burst of
dummy matmuls warms the PE clock gate (HAM) while the first chunks
are in flight.

Per-core traffic ~2.1 MB (vs 5.5 MB for the dense f32r kernel) and
PE work ~38 matmuls of 512 moving dim (~8.4 us), both near the ridge
for this shape.
"""

import sys

if "/opt/trn_rl_repo" not in sys.path:
    sys.path.insert(0, "/opt/trn_rl_repo")

import numpy as np
import ml_dtypes

B, N = 1024, 2048
P = 128
NB = N // P          # 16 spin blocks
NT = 4               # templates (J-column groups)
C = 2                # batch halves
BH = B // C          # 512 samples per core
NBLK = 34            # J blocks per core
N_WARM = 44          # dummy matmuls to warm the PE clock gate
WARM_N = 64          # free dim of each warmup matmul

# template: per column k (q = 4k+c), the d-offsets of contributing blocks
_COL_DS = [
    [0, 1, 2, 3, 4, 5, 6, 7, 8],   # k=0: 9 blocks
    [0, 1, 2, 3, 4, 5, 6, 7, 8],   # k=1: 9 blocks
    [0, 1, 2, 3, 4, 5, 6, 7],      # k=2: 8 blocks
    [0, 1, 2, 3, 4, 5, 6, 7],      # k=3: 8 blocks
]
# per-block (col k, local su slot lam = (4k+d) mod 16), in stream order
_BLOCKS = [
    (k, (4 * k + d) % NB) for k in range(4) for d in _COL_DS[k]
]
_COL_END = []  # index of last block of each column
for k in range(4):
    _COL_END.append(max(i for i, (kk, _) in enumerate(_BLOCKS) if kk == k))

_cache = {}


def _build_program():
    import concourse.bacc as bacc
    import concourse.mybir as mybir
    import concourse.tile as tile

    bf16 = mybir.dt.bfloat16
    f32 = mybir.dt.float32
    f32r = mybir.dt.float32r
    u8 = mybir.dt.uint8

    nc = bacc.Bacc("TRN2", target_bir_lowering=False, debug=False,
                   num_devices=NT * C)

    su_ext = nc.dram_tensor("su", [P, NB * BH], u8, kind="ExternalInput").ap()
    jb_ext = nc.dram_tensor("jb", [P, NBLK * P], bf16, kind="ExternalInput").ap()
    out_ext = nc.dram_tensor("part", [BH], f32, kind="ExternalOutput").ap()

    with tile.TileContext(nc) as tc:
        with (
            tc.tile_pool(name="persist", bufs=1) as persist,
            tc.tile_pool(name="work", bufs=2) as work,
            tc.tile_pool(name="psum", bufs=1, space="PSUM") as psum_pool,
            tc.tile_pool(name="warmps", bufs=1, space="PSUM") as warm_pool,
        ):
            su_t = persist.tile([P, NB, BH], u8)
            st_t = persist.tile([P, NB, BH], bf16)
            jb_t = persist.tile([P, NBLK, P], bf16)
            ones = persist.tile([P, 1], f32r)
            red_sb = persist.tile([1, BH], f32)
            warm_src = persist.tile([P, P], bf16)

            nc.vector.memset(warm_src[:], 0.0)
            # f32r memset trips an ISA check; synthesize 1.0s on the DVE
            nc.vector.tensor_scalar(
                ones[:], warm_src[:, :1], 0.0, 1.0,
                mybir.AluOpType.mult, mybir.AluOpType.add,
            )

            # PE warmup against the HAM clock gate while loads stream in
            warm_ps = warm_pool.tile([P, WARM_N], f32)
            for _ in range(N_WARM):
                nc.tensor.matmul(
                    warm_ps, lhsT=warm_src[:], rhs=warm_src[:, :WARM_N],
                    start=True, stop=True,
                )

            # Input loads, one ring (nc.sync), in exact consumption order:
            # su chunks interleaved with the J blocks they unblock.
            su3 = su_ext.rearrange("p (k b) -> p k b", b=BH)
            jb3 = jb_ext.rearrange("p (j c) -> p j c", c=P)

            def su_chunk(a, b):
                nc.sync.dma_start(out=su_t[:, a:b], in_=su3[:, a:b])

            def jb_chunk(a, b):
                nc.sync.dma_start(out=jb_t[:, a:b], in_=jb3[:, a:b])

            su_chunk(0, 4)
            jb_chunk(0, 4)     # col0 d=0..3 (slots 0..3)
            su_chunk(4, 9)
            jb_chunk(4, 9)     # col0 d=4..8 (slots 4..8)
            su_chunk(9, 13)
            jb_chunk(9, 18)    # col1 (slots 4..12)
            su_chunk(13, 16)
            jb_chunk(18, 26)   # col2 (slots 8..15)
            jb_chunk(26, 34)   # col3 (slots 12..15, 0..3)

            # expand uint8 {0,1} -> +-1.0 bf16 spins, per su slot
            for sl in range(NB):
                nc.vector.tensor_scalar(
                    st_t[:, sl],
                    su_t[:, sl],
                    2.0,
                    -1.0,
                    mybir.AluOpType.mult,
                    mybir.AluOpType.add,
                )

            ps_cols = [
                psum_pool.tile([P, BH], f32, name=f"ps_{k}") for k in range(4)
            ]
            ps_red = psum_pool.tile([1, BH], f32, name="ps_red")

            # J matmuls with epilogues interleaved.  The column-k reduce
            # matmul is deferred a few J matmuls so the PE never stalls
            # waiting on the vector-engine multiply.
            red_at = {16: 0, 24: 1, 31: 2}
            m_tiles = {}

            def epilogue_mul(k):
                m = work.tile([P, BH], f32r, name="m_col")
                nc.vector.scalar_tensor_tensor(
                    m[:],
                    ps_cols[k][:],
                    1.0,
                    st_t[:, 4 * k],
                    mybir.AluOpType.mult,
                    mybir.AluOpType.mult,
                )
                m_tiles[k] = m

            def red_mm(k):
                nc.tensor.matmul(
                    ps_red,
                    lhsT=ones[:],
                    rhs=m_tiles[k][:],
                    start=(k == 0),
                    stop=(k == 3),
                )

            seen_start = set()
            for j, (k, lam) in enumerate(_BLOCKS):
                nc.tensor.matmul(
                    ps_cols[k],
                    lhsT=jb_t[:, j],
                    rhs=st_t[:, lam],
                    start=(k not in seen_start),
                    stop=(j == _COL_END[k]),
                )
                seen_start.add(k)
                if j in _COL_END:
                    epilogue_mul(_COL_END.index(j))
                if j in red_at:
                    red_mm(red_at[j])
            red_mm(3)

            nc.scalar.copy(out=red_sb[:], in_=ps_red[:])
            nc.sync.dma_start(
                out=out_ext.rearrange("(o b) -> o b", o=1), in_=red_sb[:]
            )

    nc.compile()
    return nc


def _make_in_maps(state, J):
    bf16 = ml_dtypes.bfloat16
    state = np.asarray(state, dtype=np.float32)
    J = np.asarray(J, dtype=np.float32)

    u_all = state.astype(np.uint8)                   # [B, N] {0,1}
    uT = np.ascontiguousarray(u_all.T).reshape(NB, P, B)
    A = J + J.T                                      # symmetrized, fp32
    Ab = A.reshape(NB, P, NB, P)

    # J blocks per template c: [34, 128, 128] -> [128, 34*128] bf16
    jb_by_c = []
    for c in range(NT):
        blocks = np.empty((NBLK, P, P), dtype=np.float32)
        for j, (k, lam) in enumerate(_BLOCKS):
            q = (4 * k + c) % NB
            p = (lam + c) % NB
            w = 0.5 if p == q else 1.0
            blocks[j] = Ab[p, :, q, :] * w
        jb_by_c.append(
            np.ascontiguousarray(
                blocks.transpose(1, 0, 2).reshape(P, NBLK * P)
            ).astype(bf16)
        )

    in_maps = []
    placement = []
    for core in range(NT * C):
        c, h = divmod(core, C)
        # su slot lam holds global spin block (lam + c) mod 16
        perm = [(lam + c) % NB for lam in range(NB)]
        su = uT[perm][:, :, h * BH:(h + 1) * BH]     # [16, 128, 512]
        su = np.ascontiguousarray(
            su.transpose(1, 0, 2).reshape(P, NB * BH)
        )
        in_maps.append({"su": su, "jb": jb_by_c[c]})
        placement.append((c, h))
    return in_maps, placement


def kernel(state, J):
    from concourse.bass_utils import run_bass_kernel_spmd

    if "nc" not in _cache:
        _cache["nc"] = _build_program()
    nc = _cache["nc"]

    in_maps, placement = _make_in_maps(state, J)
    res = run_bass_kernel_spmd(nc, in_maps, list(range(NT * C)))

    out = np.zeros(B, dtype=np.float32)
    for core, (c, h) in enumerate(placement):
        out[h * BH:(h + 1) * BH] += res.results[core]["part"]
    return out


# revision 4
# speedup vs baseline: 1.4260x; 1.0877x over previous
"""Trainium2 Bass kernel: batched Ising energies E_b = s_b^T J s_b.

state: [1024, 2048] float32 in {0,1};  J: [2048, 2048] float32.
Returns energies [1024] float32.

Strategy (8 NeuronCores): symmetric-half circulant decomposition.
With A = J + J^T, E_b = 1/2 s_b^T A s_b needs only the 136 distinct
128x128 block-pairs {(p,q): p<=q} of the 16x16 block grid instead of
all 256.  A rotational starter splits those pairs into 4 isomorphic
34-block templates: core c (c = 0..3) owns block-columns
{c, 4+c, 8+c, 12+c}; column 4k+c accumulates contributions from
p = (4k+c+d) mod 16 for d = 0..7 (+ d = 8 for k = 0,1), i.e. 9/9/8/8
blocks.  Every core therefore runs the IDENTICAL instruction stream;
only the data (J blocks, spin-block permutation) differs.  The batch
is halved across the remaining factor of 2 (8 = 4 templates x 2).

Per-core compute: for column q, ps[qcol, b] = sum_p W_pq^T spins_p
via 8-9 accumulating PE matmuls (lhsT = 128x128 J block, rhs =
spins_p^T [128, 512 samples], full 512 moving dim).  Epilogue:
m = ps * spins_q^T elementwise on the vector engine (spins^T is the
same layout as the streamed state, so no second spin tensor is
shipped), then a ones-vector matmul folds the 128 partition rows into
a [1, 512] per-sample partial.  One 2KB output DMA per core; the host
sums the 4 template partials per batch half.

Scheduling notes (from perfetto traces):
 - each dma_start costs ~650 ns of DIRECT2D descriptor-write on its
   issuing sequencer, so the loads are split across TWO sequencers
   (su on gpsimd, J on sync) with small first chunks: the first
   matmul starts ~2 us earlier and the PE is never descriptor-gated.
 - uint8 -> +-1 bf16 spin expansion is split between the vector and
   scalar engines so it always stays ahead of the matmul stream.
 - the column-k reduce matmul is deferred several J matmuls so the PE
   never stalls on the vector multiply.
 - TileContext's stock teardown zeroes ~250 semaphores serially on
   gpsimd (~8 us!); _FastTeardown splits the sem_clear range across
   all five engines (~1.5 us), keeping the same drain + barriers +
   DMA-queue reset semantics.
"""

import sys

if "/opt/trn_rl_repo" not in sys.path:
    sys.path.insert(0, "/opt/trn_rl_repo")

import numpy as np
import ml_dtypes

B, N = 1024, 2048
P = 128
NB = N // P          # 16 spin blocks
NT = 4               # templates (J-column groups)
C = 2                # batch halves
BH = B // C          # 512 samples per core
NBLK = 34            # J blocks per core
N_WARM = 40          # dummy matmuls to warm the PE clock gate
WARM_N = 64          # free dim of each warmup matmul

# template: per column k (q = 4k+c), the d-offsets of contributing blocks
_COL_DS = [
    [0, 1, 2, 3, 4, 5, 6, 7, 8],   # k=0: 9 blocks
    [0, 1, 2, 3, 4, 5, 6, 7, 8],   # k=1: 9 blocks
    [0, 1, 2, 3, 4, 5, 6, 7],      # k=2: 8 blocks
    [0, 1, 2, 3, 4, 5, 6, 7],      # k=3: 8 blocks
]
# per-block (col k, local su slot lam = (4k+d) mod 16), in stream order
_BLOCKS = [
    (k, (4 * k + d) % NB) for k in range(4) for d in _COL_DS[k]
]
_COL_END = []  # index of last block of each column
for k in range(4):
    _COL_END.append(max(i for i, (kk, _) in enumerate(_BLOCKS) if kk == k))

_cache = {}


def _build_program():
    import concourse.bacc as bacc
    import concourse.mybir as mybir
    import concourse.tile as tile
    from concourse.bass import compact_to_ranges
    from concourse.vector_clock import ScopedClock

    bf16 = mybir.dt.bfloat16
    f32 = mybir.dt.float32
    f32r = mybir.dt.float32r
    u8 = mybir.dt.uint8

    class _FastTeardown(tile.TileContext):
        """Stock teardown sem_clears ~250 sems serially on gpsimd
        (~8 us).  Same semantics, but the clears are spread across all
        five engines and run concurrently between the two barriers."""

        def _drain_and_barrier(self, tick_clock, wait_clock):
            nc = self.nc
            drain_inst = nc.sync.drain()
            wait_clock.add_sem_waits(
                drain_inst.ins, ScopedClock({None: tick_clock.global_clock})
            )
            nc.all_engine_barrier()
            popped = nc._tile_sem_poison_stack.pop()
            assert popped is self._sem_poison
            sems = list(self.sems.allocated().values())
            sem_nums = [
                s.num if hasattr(s, "num") else int(s) for s in sems
            ]
            ranges = compact_to_ranges(sem_nums)
            for r in ranges:
                assert nc._state.free_isdisjoint(r)
                nc.gpsimd.dma_reset(r)
            engines = [nc.gpsimd, nc.sync, nc.scalar, nc.vector, nc.tensor]
            flat = sorted(sem_nums)
            k = (len(flat) + len(engines) - 1) // max(1, len(engines))
            for i, eng in enumerate(engines):
                part = flat[i * k:(i + 1) * k]
                for rr in compact_to_ranges(part):
                    eng.sem_clear(rr)
            nc._state.prepend_free_semaphores(sem_nums)
            for poison_set in nc._tile_sem_poison_stack:
                poison_set.update(sem_nums)
            nc.all_engine_barrier()

    nc = bacc.Bacc("TRN2", target_bir_lowering=False, debug=False,
                   num_devices=NT * C)

    su_ext = nc.dram_tensor("su", [P, NB * BH], u8, kind="ExternalInput").ap()
    jb_ext = nc.dram_tensor("jb", [P, NBLK * P], bf16, kind="ExternalInput").ap()
    out_ext = nc.dram_tensor("part", [BH], f32, kind="ExternalOutput").ap()

    with _FastTeardown(nc) as tc:
        with (
            tc.tile_pool(name="persist", bufs=1) as persist,
            tc.tile_pool(name="work", bufs=2) as work,
            tc.tile_pool(name="psum", bufs=1, space="PSUM") as psum_pool,
            tc.tile_pool(name="warmps", bufs=1, space="PSUM") as warm_pool,
        ):
            su_t = persist.tile([P, NB, BH], u8)
            st_t = persist.tile([P, NB, BH], bf16)
            jb_t = persist.tile([P, NBLK, P], bf16)
            ones = persist.tile([P, 1], f32r)
            red_sb = persist.tile([1, BH], f32)
            warm_src = persist.tile([P, P], bf16)

            nc.vector.memset(warm_src[:], 0.0)
            # f32r memset trips an ISA check; synthesize 1.0s on the DVE
            nc.vector.tensor_scalar(
                ones[:], warm_src[:, :1], 0.0, 1.0,
                mybir.AluOpType.mult, mybir.AluOpType.add,
            )

            # PE warmup against the HAM clock gate while loads stream in
            warm_ps = warm_pool.tile([P, WARM_N], f32)
            for _ in range(N_WARM):
                nc.tensor.matmul(
                    warm_ps, lhsT=warm_src[:], rhs=warm_src[:, :WARM_N],
                    start=True, stop=True,
                )

            # Input loads on TWO rings (su via gpsimd, J via sync) so the
            # ~650ns per-dma_start descriptor writes don't serialize, with
            # small head chunks so the first matmul starts early.
            su3 = su_ext.rearrange("p (k b) -> p k b", b=BH)
            jb3 = jb_ext.rearrange("p (j c) -> p j c", c=P)

            su_bounds = [0, 2, 5, 9, 12, 16]
            for a, b in zip(su_bounds[:-1], su_bounds[1:]):
                nc.gpsimd.dma_start(out=su_t[:, a:b], in_=su3[:, a:b])
            jb_bounds = [0, 2, 5, 9, 14, 18, 26, 34]
            for a, b in zip(jb_bounds[:-1], jb_bounds[1:]):
                nc.sync.dma_start(out=jb_t[:, a:b], in_=jb3[:, a:b])

            # expand uint8 {0,1} -> +-1.0 bf16 spins, one op per slot;
            # slots 0-8 on the vector engine, 9-15 on the scalar engine
            for sl in range(9):
                nc.vector.tensor_scalar(
                    st_t[:, sl], su_t[:, sl], 2.0, -1.0,
                    mybir.AluOpType.mult, mybir.AluOpType.add,
                )
            for sl in range(9, NB):
                nc.scalar.activation(
                    st_t[:, sl], su_t[:, sl],
                    mybir.ActivationFunctionType.Copy,
                    bias=-1.0, scale=2.0,
                )

            ps_cols = [
                psum_pool.tile([P, BH], f32, name=f"ps_{k}") for k in range(4)
            ]
            ps_red = psum_pool.tile([1, BH], f32, name="ps_red")

            # J matmuls with epilogues interleaved.  The column-k reduce
            # matmul is deferred so the PE never stalls on the DVE multiply.
            red_at = {20: 0, 26: 1, 31: 2}
            m_tiles = {}

            def epilogue_mul(k):
                m = work.tile([P, BH], f32r, name="m_col")
                nc.vector.scalar_tensor_tensor(
                    m[:],
                    ps_cols[k][:],
                    1.0,
                    st_t[:, 4 * k],
                    mybir.AluOpType.mult,
                    mybir.AluOpType.mult,
                )
                m_tiles[k] = m

            def red_mm(k):
                nc.tensor.matmul(
                    ps_red,
                    lhsT=ones[:],
                    rhs=m_tiles[k][:],
                    start=(k == 0),
                    stop=(k == 3),
                )

            seen_start = set()
            for j, (k, lam) in enumerate(_BLOCKS):
                nc.tensor.matmul(
                    ps_cols[k],
                    lhsT=jb_t[:, j],
                    rhs=st_t[:, lam],
                    start=(k not in seen_start),
                    stop=(j == _COL_END[k]),
                )
                seen_start.add(k)
                if j in _COL_END:
                    epilogue_mul(_COL_END.index(j))
                if j in red_at:
                    red_mm(red_at[j])
            red_mm(3)

            nc.scalar.copy(out=red_sb[:], in_=ps_red[:])
            nc.sync.dma_start(
                out=out_ext.rearrange("(o b) -> o b", o=1), in_=red_sb[:]
            )

    nc.compile()
    return nc


def _make_in_maps(state, J):
    bf16 = ml_dtypes.bfloat16
    state = np.asarray(state, dtype=np.float32)
    J = np.asarray(J, dtype=np.float32)

    u_all = state.astype(np.uint8)                   # [B, N] {0,1}
    uT = np.ascontiguousarray(u_all.T).reshape(NB, P, B)
    A = J + J.T                                      # symmetrized, fp32
    Ab = A.reshape(NB, P, NB, P)

    # J blocks per template c: [34, 128, 128] -> [128, 34*128] bf16
    jb_by_c = []
    for c in range(NT):
        blocks = np.empty((NBLK, P, P), dtype=np.float32)
        for j, (k, lam) in enumerate(_BLOCKS):
            q = (4 * k + c) % NB
            p = (lam + c) % NB
            w = 0.5 if p == q else 1.0
            blocks[j] = Ab[p, :, q, :] * w
        jb_by_c.append(
            np.ascontiguousarray(
                blocks.transpose(1, 0, 2).reshape(P, NBLK * P)
            ).astype(bf16)
        )

    in_maps = []
    placement = []
    for core in range(NT * C):
        c, h = divmod(core, C)
        # su slot lam holds global spin block (lam + c) mod 16
        perm = [(lam + c) % NB for lam in range(NB)]
        su = uT[perm][:, :, h * BH:(h + 1) * BH]     # [16, 128, 512]
        su = np.ascontiguousarray(
            su.transpose(1, 0, 2).reshape(P, NB * BH)
        )
        in_maps.append({"su": su, "jb": jb_by_c[c]})
        placement.append((c, h))
    return in_maps, placement


def kernel(state, J):
    from concourse.bass_utils import run_bass_kernel_spmd

    if "nc" not in _cache:
        _cache["nc"] = _build_program()
    nc = _cache["nc"]

    in_maps, placement = _make_in_maps(state, J)
    res = run_bass_kernel_spmd(nc, in_maps, list(range(NT * C)))

    out = np.zeros(B, dtype=np.float32)
    for core, (c, h) in enumerate(placement):
        out[h * BH:(h + 1) * BH] += res.results[core]["part"]
    return out
